# revision 41
# baseline (speedup 1.0000x reference)
"""Trainium2 Bass kernel for nn_Decoder_offset001 (dense CNN decoder with
deformable convs), data-parallel over 8 NeuronCores.

v2: bf16 datapath (PSUM fp32), DMA-broadcast field replication (no
stream_shuffle), PSUM-direct tap accumulation for part of the deform taps,
tap-major interleaved matmul emission to avoid PSUM RAW stalls.

Sharding: 8 shards = 2 batches x 4 H-strips of 64 output rows, each strip
carrying a 14-row halo (92 rows, zero-padded at image borders) and 1-col
zero pads (258 wide).  Each core runs the full network on its strip; host
gathers the central 64 rows.

Key math: every bilinear sample lands within +-1 px of its output pixel
(|offset| = 0.08|randn| < 1), so deform_conv(x) = sum_k Wd_k.T @ s_k with
s_k = sum_rc P^k_rc (.) shift_rc(x), P^k_rc = wy_r * wx_c per-pixel fields
computed on host.  Fields stream in compact [32(sq),4(g),3(tg),WP] tiles;
an SBUF->SBUF broadcast DMA replicates each field row across the 32
channel partitions of all four row-quarters.  Products are built on DVE;
tap accumulation is split between DVE adds, GPSIMD adds, and PSUM-direct
matmuls (PE absorbs the sum for PE_KS kernel points).
"""
import sys
import numpy as np
import ml_dtypes

for _p in ('/opt/trn_rl_repo',):
    if _p not in sys.path:
        sys.path.insert(0, _p)

BF = ml_dtypes.bfloat16

RATIO = 0.08
GX = np.repeat(np.arange(-1, 2), 3)
GY = np.tile(np.arange(-1, 2), 3)
RC = [(r, c) for r in (-1, 0, 1) for c in (-1, 0, 1)]

HALO = 14
ROWS = 92
W = 256
WP = 258
SR = 52                  # 64-ch slab rows (local 0..51)
SOFF = 40                # slab1 strip-row offset
FROWS = 50               # front conv out-rows local 1..50
QR = 28                  # 32-ch quarter rows
QOFF = [8 + 16 * g for g in range(4)]
NSTEP = 26               # deform out-rows local 1..26
RING = 5

# deform tap-accumulation modes per kernel point k:
#   k in PE_KS  -> every tap product matmul'd straight into PSUM
#   k in GPS_KS -> tap adds chained on GPSIMD
#   else        -> tap adds chained on DVE
PE_KS = (0, 2, 4, 6, 8, 5, 1, 3, 7)
GPS_KS = ()

_cache = {}


def split_excess_waits(nc, mybir):
    """Walrus here allows 1 sync-wait per instruction (2 for EventSemaphore);
    Tile emits more.  Move excess waits onto inserted same-engine NOPs."""
    n = 0
    for bbh in nc.bb_map.values():
        bb = bbh.bb
        out, changed = [], False
        for inst in bb.instructions:
            si = inst.sync_info
            cap = 2 if isinstance(inst, mybir.InstEventSemaphore) else 1
            if si is not None and si.on_wait is not None and len(si.on_wait) > cap:
                waits = list(si.on_wait)
                extra, keep = waits[:-cap], waits[-cap:]
                for w_ in extra:
                    nop = mybir.InstNoOp(
                        name=nc.get_next_instruction_name(),
                        engine=inst.engine, ins=[], outs=[],
                        sync_info=mybir.SyncInfo(on_wait=[w_], on_update=[]))
                    nc.register_instruction(nop)
                    out.append(nop)
                    n += 1
                inst.sync_info = mybir.SyncInfo(on_wait=keep,
                                                on_update=si.on_update)
                changed = True
            out.append(inst)
        if changed:
            bb.instructions = out
    return n


def build_nc():
    import concourse.bass as bass
    import concourse.mybir as mybir
    import concourse.tile as tile
    from contextlib import ExitStack

    f32 = mybir.dt.float32
    bf16 = mybir.dt.bfloat16
    AF = mybir.ActivationFunctionType
    ALU = mybir.AluOpType

    nc = bass.Bass()
    xin = nc.declare_dram_parameter("xin", [64, ROWS, WP], bf16, isOutput=False)
    flds = nc.declare_dram_parameter("flds", [7, 128, 3, 4, W], bf16,
                                     isOutput=False)
    wcv = nc.declare_dram_parameter("wcv", [128, 5 * 9 * 128], bf16, isOutput=False)
    w15 = nc.declare_dram_parameter("w15", [128, 9 * 32], bf16, isOutput=False)
    wdf = nc.declare_dram_parameter("wdf", [128, 4 * 9 * 128], bf16, isOutput=False)
    w24 = nc.declare_dram_parameter("w24", [128, 9 * 3], bf16, isOutput=False)
    b24 = nc.declare_dram_parameter("b24", [128, 1], f32, isOutput=False)
    maskc = nc.declare_dram_parameter("maskc", [128, FROWS], f32, isOutput=False)
    mbiasc = nc.declare_dram_parameter("mbiasc", [128, 5 * FROWS], f32, isOutput=False)
    mq15 = nc.declare_dram_parameter("mq15", [128, QR], f32, isOutput=False)
    mb15 = nc.declare_dram_parameter("mb15", [128, QR], f32, isOutput=False)
    maskq = nc.declare_dram_parameter("maskq", [128, NSTEP], f32, isOutput=False)
    mbiasq = nc.declare_dram_parameter("mbiasq", [128, 4 * NSTEP], f32, isOutput=False)
    out = nc.declare_dram_parameter("out", [3, 64, W], f32, isOutput=True)

    with ExitStack() as ctx:
        tc = ctx.enter_context(tile.TileContext(nc))
        wp_ = ctx.enter_context(tc.tile_pool(name="w", bufs=1))
        big = ctx.enter_context(tc.tile_pool(name="big", bufs=1))
        qp = ctx.enter_context(tc.tile_pool(name="q", bufs=1))
        fr = ctx.enter_context(tc.tile_pool(name="fld", bufs=4))
        rp = ctx.enter_context(tc.tile_pool(name="rep", bufs=16))
        sp = ctx.enter_context(tc.tile_pool(name="s", bufs=16))
        se = ctx.enter_context(tc.tile_pool(name="se", bufs=4))
        ppF = ctx.enter_context(tc.tile_pool(name="psF", bufs=4, space="PSUM"))
        ppD = ctx.enter_context(tc.tile_pool(name="psD", bufs=4, space="PSUM"))

        def psum_tile(pool, tag):
            pst = pool.tile([128, 512], f32, tag=tag, name=tag)
            return pst[:, 0:W]

        def load(tag, param, cols, dt):
            t = wp_.tile([128, cols], dt, tag=tag, name=tag)
            nc.sync.dma_start(t[:], param[:, :])
            return t

        wcv_t = load("wcv", wcv, 5 * 9 * 128, bf16)
        w15_t = load("w15", w15, 9 * 32, bf16)
        wdf_t = load("wdf", wdf, 4 * 9 * 128, bf16)
        w24_t = load("w24", w24, 9 * 3, bf16)
        b24_t = load("b24", b24, 1, f32)
        mkc_t = load("mkc", maskc, FROWS, f32)
        mbc_t = load("mbc", mbiasc, 5 * FROWS, f32)
        mq15_t = load("mq15t", mq15, QR, f32)
        mb15_t = load("mb15t", mb15, QR, f32)
        mkq_t = load("mkq", maskq, NSTEP, f32)
        mbq_t = load("mbq", mbiasq, 4 * NSTEP, f32)

        def wcv_ap(stage, k):
            return wcv_t[:, (stage * 9 + k) * 128:(stage * 9 + k + 1) * 128]

        def wdf_ap(d, k):
            return wdf_t[:, (d * 9 + k) * 128:(d * 9 + k + 1) * 128]

        # ---- x input ring ----
        xr = big.tile([128, 4, WP], bf16, tag="xring")
        for s in (0, 1, 2):
            nc.sync.dma_start(xr[0:64, s, :], xin[:, s, :])
            nc.sync.dma_start(xr[64:128, s, :], xin[:, SOFF + s, :])

        # ---- one big 64-ch tile (T1 -> T2 -> T3 in place) ----
        T = big.tile([128, SR, WP], bf16, tag="T")
        nc.gpsimd.memset(T[:, 0, :], 0.0)
        nc.gpsimd.memset(T[:, SR - 1, :], 0.0)
        nc.gpsimd.memset(T[:, 1:SR - 1, 0:1], 0.0)
        nc.gpsimd.memset(T[:, 0:SR - 1, WP - 1:WP], 0.0)
        u1 = big.tile([128, RING, WP], bf16, tag="u1")
        nc.gpsimd.memset(u1[:], 0.0)
        u2 = big.tile([128, RING, WP], bf16, tag="u2")
        nc.gpsimd.memset(u2[:], 0.0)

        def evict_resid(dst_ap, ps, mb_ap, m_ap, resid_ap):
            t = se.tile([128, W], bf16, tag="ev", name="ev")
            nc.scalar.activation(t[:], ps[:], AF.Identity, bias=mb_ap, scale=m_ap)
            nc.vector.tensor_tensor(dst_ap, t[:], resid_ap, ALU.add)

        # tap-major interleaved front matmuls for a set of wavefront jobs
        def mm_multi(jobs):
            # jobs: list of (ps, stage, src_rows, skipfn or None)
            plans = []
            for (ps, stage, src_rows, skip) in jobs:
                taps = [(k, r, c) for k, (r, c) in enumerate(RC)
                        if skip is None or skip(r)]
                plans.append((ps, stage, src_rows, taps))
            for idx in range(9):
                for (ps, stage, src_rows, taps) in plans:
                    if idx >= len(taps):
                        continue
                    k, r, c = taps[idx]
                    last = (idx == len(taps) - 1)
                    nc.tensor.matmul(
                        ps[:, :], wcv_ap(stage, k),
                        src_rows(r)[:, 1 + c:1 + c + W],
                        start=(idx == 0), stop=last,
                        skip_group_check=True)

        # ---------------- front stack, fused wavefront ----------------
        # Stage lags (0,2,4,6,8): a stage's matmuls are emitted before this
        # iteration's evictions, so readers must only touch rows evicted in
        # PRIOR iterations.
        for i in range(1, FROWS + 9):
            if 3 <= i + 1 <= FROWS + 1:
                nc.sync.dma_start(xr[0:64, (i + 1) % 4, :], xin[:, i + 1, :])
                nc.sync.dma_start(xr[64:128, (i + 1) % 4, :],
                                  xin[:, SOFF + i + 1, :])
            jobs = []
            evs = []
            if i <= FROWS:
                ps = psum_tile(ppF, "psF")
                jobs.append((ps, 0, (lambda i=i: lambda r: xr[:, (i + r) % 4, :])(),
                             None))
                evs.append(('act', ps, i, 0, T[:, i, 1:1 + W], AF.Identity))
            m = i - 2
            if 1 <= m <= FROWS:
                ps = psum_tile(ppF, "psF")
                jobs.append((ps, 1, (lambda m=m: lambda r: T[:, m + r, :])(), None))
                evs.append(('act', ps, m, 1, u1[:, m % RING, 1:1 + W], AF.Relu))
            m = i - 4
            if 1 <= m <= FROWS:
                ps = psum_tile(ppF, "psF")
                jobs.append((ps, 2,
                             (lambda m=m: lambda r: u1[:, (m + r) % RING, :])(),
                             (lambda m=m: lambda r: 1 <= m + r <= FROWS)()))
                evs.append(('resid', ps, m, 2, T[:, m, 1:1 + W], None))
            m = i - 6
            if 1 <= m <= FROWS:
                ps = psum_tile(ppF, "psF")
                jobs.append((ps, 3, (lambda m=m: lambda r: T[:, m + r, :])(), None))
                evs.append(('act', ps, m, 3, u2[:, m % RING, 1:1 + W], AF.Relu))
            m = i - 8
            if 1 <= m <= FROWS:
                ps = psum_tile(ppF, "psF")
                jobs.append((ps, 4,
                             (lambda m=m: lambda r: u2[:, (m + r) % RING, :])(),
                             (lambda m=m: lambda r: 1 <= m + r <= FROWS)()))
                evs.append(('resid', ps, m, 4, T[:, m, 1:1 + W], None))
            mm_multi(jobs)
            for (kind, ps, m, stage, dst, af) in evs:
                mb = mbc_t[:, stage * FROWS + m - 1:stage * FROWS + m]
                mk = mkc_t[:, m - 1:m]
                if kind == 'act':
                    nc.scalar.activation(dst, ps[:], af, bias=mb, scale=mk)
                else:
                    evict_resid(dst, ps, mb, mk, dst)

        # ---------------- l15: 64 -> 32 into quarter tile ----------------
        XQ = qp.tile([128, QR, WP], bf16, tag="XQ")
        nc.gpsimd.memset(XQ[:, :, 0:1], 0.0)
        nc.gpsimd.memset(XQ[:, :, WP - 1:WP], 0.0)
        for dj in range(QR):
            ps = psum_tile(ppF, "psF")
            for k, (r, c) in enumerate(RC):
                for g in range(4):
                    j = QOFF[g] + dj
                    s = 0 if j <= 45 else 1
                    rl = j - (0 if s == 0 else SOFF)
                    nc.tensor.matmul(
                        ps[32 * g:32 * g + 32, :],
                        w15_t[64 * s:64 * s + 64, k * 32:(k + 1) * 32],
                        T[64 * s:64 * s + 64, rl + r, 1 + c:1 + c + W],
                        start=(k == 0), stop=(k == 8),
                        tile_position=(64 * s, 32 * g), skip_group_check=True)
            nc.scalar.activation(XQ[:, dj, 1:1 + W], ps[:], AF.Identity,
                                 bias=mb15_t[:, dj:dj + 1],
                                 scale=mq15_t[:, dj:dj + 1])

        # ---------------- deform conv pairs (2-row steps) ----------------
        # Full-size relu tiles (28 rows) instead of rings: rows 0 and 27 stay
        # zero forever, so out-of-range taps read zeros (== skipping them) and
        # no per-tap skip logic or ring aliasing exists.
        NPE = len(PE_KS)

        def deform_rows_multi(jobs):
            # jobs co-emitted with tap loops interleaved: their PSUM tiles
            # rotate (spacing 4) so accumulation RAW latency is hidden.
            st = []
            for (d, lj0, ft, srcT, relu, dstT, nr) in jobs:
                npair = nr // 2
                pss = [ppD.tile([128, 512], f32, tag="psD", name="psD")
                       for _ in range(npair)]
                st.append({'d': d, 'lj0': lj0, 'ft': ft, 'srcT': srcT,
                           'relu': relu, 'dstT': dstT, 'nr': nr,
                           'npair': npair, 'pss': pss, 'nmm': [0] * npair})
            for k in (1, 3, 7, 0, 2, 4, 5, 6, 8):
                prods = {id(j): [] for j in st}
                for i2, (r, c) in enumerate(RC):
                    t_ = k * 9 + i2
                    sq, tg = t_ % 32, t_ // 32
                    for j in st:
                        nr = j['nr']
                        lj0 = j['lj0']
                        rep = rp.tile([128, 4, W], bf16, tag="rep", name="rep")
                        nc.vector.stream_shuffle(
                            rep[:, 0:nr, :].bitcast(f32),
                            j['ft'][:, tg, 0:nr, :].bitcast(f32), [sq] * 32)
                        prod = sp.tile([128, 4, W], bf16, tag="prod",
                                       name="prod")
                        nc.vector.tensor_tensor(
                            prod[:, 0:nr, :], rep[:, 0:nr, :],
                            j['srcT'][:, lj0 + r:lj0 + r + nr,
                                      1 + c:1 + c + W], ALU.mult)
                        prods[id(j)].append(prod)
                # same-weight matmuls grouped: one LDWEIGHTS stretch per job/k
                for j in st:
                    wap = wdf_ap(j['d'], k)
                    for prod in prods[id(j)]:
                        for pp in range(j['npair']):
                            nc.tensor.matmul(
                                j['pss'][pp][:, :], wap,
                                prod[:, 2 * pp:2 * pp + 2, :],
                                start=(j['nmm'][pp] == 0),
                                stop=(j['nmm'][pp] == 80),
                                skip_group_check=True)
                            j['nmm'][pp] += 1
            for j in st:
                for p in range(j['nr']):
                    lj = j['lj0'] + p
                    mb = mbq_t[:, j['d'] * NSTEP + lj - 1:j['d'] * NSTEP + lj]
                    mk = mkq_t[:, lj - 1:lj]
                    half = j['pss'][p // 2][:, (p % 2) * W:(p % 2 + 1) * W]
                    if j['relu']:
                        nc.scalar.activation(j['dstT'][:, lj, 1:1 + W], half,
                                             AF.Relu, bias=mb, scale=mk)
                    else:
                        evict_resid(XQ[:, lj, 1:1 + W], half, mb, mk,
                                    XQ[:, lj, 1:1 + W])

        STEPS = [(1, 4), (5, 4), (9, 4), (13, 4), (17, 4), (21, 4), (25, 2)]

        def deform_pair(d_relu, d_resid, ring, after_resid=None):
            # resid lags the relu wavefront by TWO slots so its ring reads
            # (rows up to lj0+nr) are final before co-emission.
            fts = {}
            for si in range(len(STEPS) + 2):
                jobs = []
                if si < len(STEPS):
                    lj0, nr = STEPS[si]
                    ft = fr.tile([128, 3, 4, W], bf16, tag="fld", name="fld")
                    nc.sync.dma_start(ft[:], flds[si])
                    fts[si] = ft
                    jobs.append((d_relu, lj0, ft, XQ, True, ring, nr))
                if si >= 2:
                    lj0, nr = STEPS[si - 2]
                    jobs.append((d_resid, lj0, fts.pop(si - 2), ring, False,
                                 None, nr))
                if jobs:
                    deform_rows_multi(jobs)
                    if si >= 2 and after_resid is not None:
                        after_resid(STEPS[si - 2][0] + STEPS[si - 2][1] - 1)

        def emit_l24_group(b):
            jo0 = 4 * b
            ob = se.tile([128, 4 * W], f32, tag="ob", name="ob")
            pss = []
            gs = []
            for j4 in range(4):
                jo = jo0 + j4
                g = min(jo // 16, 3)
                pss.append(psum_tile(ppF, "psF"))
                gs.append(g)
            for k, (r, c) in enumerate(RC):
                for j4 in range(4):
                    jo = jo0 + j4
                    g = gs[j4]
                    dj = jo + HALO - QOFF[g]
                    nc.tensor.matmul(
                        pss[j4][0:3, :],
                        w24_t[32 * g:32 * g + 32, k * 3:(k + 1) * 3],
                        XQ[32 * g:32 * g + 32, dj + r, 1 + c:1 + c + W],
                        start=(k == 0), stop=(k == 8),
                        tile_position=(32 * g, 0), skip_group_check=True)
            for j4 in range(4):
                nc.scalar.activation(ob[0:3, j4 * W:(j4 + 1) * W],
                                     pss[j4][0:3, :], AF.Identity,
                                     bias=b24_t[0:3, :])
            nc.sync.dma_start(
                out[:, jo0:jo0 + 4, :],
                ob[0:3, :].rearrange("p (a b) -> p a b", a=4))

        done_b = set()

        def l24_after(R):
            for b in range(16):
                if b not in done_b and 4 * (b % 4) + 10 <= R:
                    done_b.add(b)
                    emit_l24_group(b)

        r5 = qp.tile([128, QR, WP], bf16, tag="r5")
        nc.gpsimd.memset(r5[:], 0.0)
        deform_pair(0, 1, r5)
        r6 = qp.tile([128, QR, WP], bf16, tag="r6")
        nc.gpsimd.memset(r6[:], 0.0)
        deform_pair(2, 3, r6, after_resid=l24_after)
        l24_after(99)

    import concourse.mybir as mybir2
    split_excess_waits(nc, mybir2)
    return nc


# ----------------------------------------------------------------------------
# host side
# ----------------------------------------------------------------------------
def _lhsT_dup2(w, co):
    o = np.empty((9, 128, co), np.float32)
    for k, (r, c) in enumerate(RC):
        l = np.ascontiguousarray(w[:, :, r + 1, c + 1].T)
        o[k, 0:64] = l
        o[k, 64:128] = l
    return o


def _lhsT_bd2(w):
    # block-diagonal [128,128]: both 64-ch slabs in one matmul
    o = np.zeros((9, 128, 128), np.float32)
    for k, (r, c) in enumerate(RC):
        l = w[:, :, r + 1, c + 1].T
        o[k, 0:64, 0:64] = l
        o[k, 64:128, 64:128] = l
    return o


def _lhsT_dup4(w, co, grid=False):
    o = np.empty((9, 128, co), np.float32)
    for k in range(9):
        if grid:
            l = w[:, :, GY[k] + 1, GX[k] + 1].T
        else:
            r, c = RC[k]
            l = w[:, :, r + 1, c + 1].T
        for g in range(4):
            o[k, 32 * g:32 * g + 32] = l
    return o


def _flat_w(stack):
    """[S, 9, 128, co] or [9, 128, co] -> [128, S*9*co]"""
    a = np.asarray(stack, np.float32)
    if a.ndim == 3:
        a = a[None]
    return np.ascontiguousarray(a.transpose(2, 0, 1, 3).reshape(128, -1))


def _strip(a, r0, rows):
    C, H, _ = a.shape
    t = np.zeros((C, rows, WP), np.float32)
    lo, hi = max(r0, 0), min(r0 + rows, H)
    if hi > lo:
        t[:, lo - r0:hi - r0, 1:1 + W] = a[:, lo:hi]
    return t


def _prep_shards(inputs):
    x = np.asarray(inputs['x'], np.float32)
    off = np.asarray(inputs['offset_0'], np.float32)
    B, C, H, Wi = x.shape

    wcv = _flat_w(np.stack([_lhsT_bd2(np.asarray(inputs[n], np.float32))
                            for n in ('l12_w', 'l13_w1', 'l13_w2',
                                      'l14_w1', 'l14_w2')])).astype(BF)
    w15a = _flat_w(_lhsT_dup2(np.asarray(inputs['l15_w'], np.float32), 32)).astype(BF)
    def _lhsT_bd4(w):
        o = np.zeros((9, 128, 128), np.float32)
        for k in range(9):
            l = w[:, :, GY[k] + 1, GX[k] + 1].T
            for g in range(4):
                o[k, 32 * g:32 * g + 32, 32 * g:32 * g + 32] = l
        return o

    wdf = _flat_w(np.stack([_lhsT_bd4(np.asarray(inputs[n], np.float32))
                            for n in ('d50_w', 'd51_w', 'd60_w', 'd61_w')])).astype(BF)
    w24a = _flat_w(_lhsT_dup4(np.asarray(inputs['l24_w'], np.float32), 3)).astype(BF)
    b24 = np.zeros((128, 1), np.float32)
    b24[0:3, 0] = np.asarray(inputs['l24_b'], np.float32)

    fb = {k: np.asarray(inputs[k], np.float32) for k in
          ('l12_b', 'l13_b1', 'l13_b2', 'l14_b1', 'l14_b2', 'l15_b',
           'd50_b', 'd51_b', 'd60_b', 'd61_b')}

    shards = []
    for b in range(B):
        ov = off[b].reshape(12, 2, H, Wi)
        crop = ov[3:12]
        dxs_f = crop[:, 0] * RATIO
        dys_f = crop[:, 1] * RATIO
        for g4 in range(4):
            r0 = g4 * 64 - HALO

            def m(sr):
                return np.float32(1.0 if 0 <= r0 + sr < H else 0.0)

            xin = _strip(x[b], r0, ROWS).astype(BF)

            dxs = _strip(dxs_f, r0, ROWS)
            dys = _strip(dys_f, r0, ROWS)
            ax, ay = np.abs(dxs), np.abs(dys)
            wx3 = np.stack([(ax - dxs) * .5, 1 - ax, (dxs + ax) * .5])
            wy3 = np.stack([(ay - dys) * .5, 1 - ay, (dys + ay) * .5])
            fl = np.zeros((NSTEP, 128, 3, WP), np.float32)
            for k in range(9):
                for i2, (r, c) in enumerate(RC):
                    t = k * 9 + i2
                    sq, tg = t % 32, t // 32
                    P = wy3[r + 1, k] * wx3[c + 1, k]      # [ROWS, WP]
                    for qg in range(4):
                        base = QOFF[qg] + 1
                        fl[:, 32 * qg + sq, tg, :] = P[base:base + NSTEP, :]
            # per-step tile layout [7, 128, 3, 4, WP] (4-row steps, 2-row tail)
            fl4 = np.zeros((7, 128, 3, 4, W), np.float32)
            for si, (lj0, nr) in enumerate(
                    [(1, 4), (5, 4), (9, 4), (13, 4), (17, 4), (21, 4),
                     (25, 2)]):
                fl4[si, :, :, 0:nr, :] = \
                    fl[lj0 - 1:lj0 - 1 + nr, :, :, 1:1 + W].transpose(
                        1, 2, 0, 3)
            fl = fl4

            mkc = np.zeros((128, FROWS), np.float32)
            for i2 in range(1, FROWS + 1):
                mkc[0:64, i2 - 1] = m(i2)
                mkc[64:128, i2 - 1] = m(SOFF + i2)
            mbc = np.zeros((128, 5 * FROWS), np.float32)
            for si, nm in enumerate(('l12_b', 'l13_b1', 'l13_b2',
                                     'l14_b1', 'l14_b2')):
                col = np.concatenate([fb[nm], fb[nm]])
                mbc[:, si * FROWS:(si + 1) * FROWS] = mkc * col[:, None]
            mq = np.zeros((128, QR), np.float32)
            for dj in range(QR):
                for qg in range(4):
                    mq[32 * qg:32 * qg + 32, dj] = m(QOFF[qg] + dj)
            mb15v = mq * np.tile(fb['l15_b'], 4)[:, None]
            mkq = np.zeros((128, NSTEP), np.float32)
            for jj in range(NSTEP):
                for qg in range(4):
                    mkq[32 * qg:32 * qg + 32, jj] = m(QOFF[qg] + 1 + jj)
            mbq = np.zeros((128, 4 * NSTEP), np.float32)
            for di, nm in enumerate(('d50_b', 'd51_b', 'd60_b', 'd61_b')):
                mbq[:, di * NSTEP:(di + 1) * NSTEP] = \
                    mkq * np.tile(fb[nm], 4)[:, None]

            shards.append({
                'xin': xin, 'flds': fl.astype(BF), 'wcv': wcv, 'w15': w15a,
                'wdf': wdf, 'w24': w24a, 'b24': b24, 'maskc': mkc,
                'mbiasc': mbc, 'mq15': mq, 'mb15': mb15v, 'maskq': mkq,
                'mbiasq': mbq,
            })
    return shards


def kernel(**inputs):
    if 'nc' not in _cache:
        _cache['nc'] = build_nc()
    from concourse.bass_utils import run_bass_kernel_spmd
    shards = _prep_shards(inputs)
    res = run_bass_kernel_spmd(_cache['nc'], shards, core_ids=list(range(8)))
    out = np.empty((2, 3, 256, 256), np.float32)
    for i in range(8):
        b, g = divmod(i, 4)
        out[b, :, g * 64:(g + 1) * 64, :] = res.results[i]['out']
    return out


# revision 42
# speedup vs baseline: 1.1974x; 1.1974x over previous
"""Trainium2 Bass kernel for nn_Decoder_offset001 (dense CNN decoder with
deformable convs), data-parallel over 8 NeuronCores.

v2: bf16 datapath (PSUM fp32), DMA-broadcast field replication (no
stream_shuffle), PSUM-direct tap accumulation for part of the deform taps,
tap-major interleaved matmul emission to avoid PSUM RAW stalls.

Sharding: 8 shards = 2 batches x 4 H-strips of 64 output rows, each strip
carrying a 14-row halo (92 rows, zero-padded at image borders) and 1-col
zero pads (258 wide).  Each core runs the full network on its strip; host
gathers the central 64 rows.

Key math: every bilinear sample lands within +-1 px of its output pixel
(|offset| = 0.08|randn| < 1), so deform_conv(x) = sum_k Wd_k.T @ s_k with
s_k = sum_rc P^k_rc (.) shift_rc(x), P^k_rc = wy_r * wx_c per-pixel fields
computed on host.  Fields stream in compact [32(sq),4(g),3(tg),WP] tiles;
an SBUF->SBUF broadcast DMA replicates each field row across the 32
channel partitions of all four row-quarters.  Products are built on DVE;
tap accumulation is split between DVE adds, GPSIMD adds, and PSUM-direct
matmuls (PE absorbs the sum for PE_KS kernel points).
"""
import sys
import numpy as np
import ml_dtypes

for _p in ('/opt/trn_rl_repo',):
    if _p not in sys.path:
        sys.path.insert(0, _p)

BF = ml_dtypes.bfloat16

RATIO = 0.08
GX = np.repeat(np.arange(-1, 2), 3)
GY = np.tile(np.arange(-1, 2), 3)
RC = [(r, c) for r in (-1, 0, 1) for c in (-1, 0, 1)]

HALO = 14
ROWS = 92
W = 256
WP = 258
SR = 52                  # 64-ch slab rows (local 0..51)
SOFF = 40                # slab1 strip-row offset
FROWS = 50               # front conv out-rows local 1..50
QR = 28                  # 32-ch quarter rows
QOFF = [8 + 16 * g for g in range(4)]
NSTEP = 26               # deform out-rows local 1..26
RING = 5

# deform tap-accumulation modes per kernel point k:
#   k in PE_KS  -> every tap product matmul'd straight into PSUM
#   k in GPS_KS -> tap adds chained on GPSIMD
#   else        -> tap adds chained on DVE
PE_KS = (0, 2, 4, 6, 8, 5, 1, 3, 7)
GPS_KS = ()

_cache = {}


def split_excess_waits(nc, mybir):
    """Walrus here allows 1 sync-wait per instruction (2 for EventSemaphore);
    Tile emits more.  Move excess waits onto inserted same-engine NOPs."""
    n = 0
    for bbh in nc.bb_map.values():
        bb = bbh.bb
        out, changed = [], False
        for inst in bb.instructions:
            si = inst.sync_info
            cap = 2 if isinstance(inst, mybir.InstEventSemaphore) else 1
            if si is not None and si.on_wait is not None and len(si.on_wait) > cap:
                waits = list(si.on_wait)
                extra, keep = waits[:-cap], waits[-cap:]
                for w_ in extra:
                    nop = mybir.InstNoOp(
                        name=nc.get_next_instruction_name(),
                        engine=inst.engine, ins=[], outs=[],
                        sync_info=mybir.SyncInfo(on_wait=[w_], on_update=[]))
                    nc.register_instruction(nop)
                    out.append(nop)
                    n += 1
                inst.sync_info = mybir.SyncInfo(on_wait=keep,
                                                on_update=si.on_update)
                changed = True
            out.append(inst)
        if changed:
            bb.instructions = out
    return n


def build_nc():
    import concourse.bass as bass
    import concourse.mybir as mybir
    import concourse.tile as tile
    from contextlib import ExitStack

    f32 = mybir.dt.float32
    bf16 = mybir.dt.bfloat16
    AF = mybir.ActivationFunctionType
    ALU = mybir.AluOpType

    nc = bass.Bass()
    xin = nc.declare_dram_parameter("xin", [64, ROWS, WP], bf16, isOutput=False)
    flds = nc.declare_dram_parameter("flds", [7, 128, 3, 4, W], bf16,
                                     isOutput=False)
    wcv = nc.declare_dram_parameter("wcv", [128, 5 * 9 * 128], bf16, isOutput=False)
    w15 = nc.declare_dram_parameter("w15", [128, 9 * 32], bf16, isOutput=False)
    wdf = nc.declare_dram_parameter("wdf", [128, 4 * 9 * 128], bf16, isOutput=False)
    w24 = nc.declare_dram_parameter("w24", [128, 9 * 3], bf16, isOutput=False)
    b24 = nc.declare_dram_parameter("b24", [128, 1], f32, isOutput=False)
    maskc = nc.declare_dram_parameter("maskc", [128, FROWS], f32, isOutput=False)
    mbiasc = nc.declare_dram_parameter("mbiasc", [128, 5 * FROWS], f32, isOutput=False)
    mq15 = nc.declare_dram_parameter("mq15", [128, QR], f32, isOutput=False)
    mb15 = nc.declare_dram_parameter("mb15", [128, QR], f32, isOutput=False)
    maskq = nc.declare_dram_parameter("maskq", [128, NSTEP], f32, isOutput=False)
    mbiasq = nc.declare_dram_parameter("mbiasq", [128, 4 * NSTEP], f32, isOutput=False)
    out = nc.declare_dram_parameter("out", [3, 64, W], f32, isOutput=True)

    with ExitStack() as ctx:
        tc = ctx.enter_context(tile.TileContext(nc))
        wp_ = ctx.enter_context(tc.tile_pool(name="w", bufs=1))
        big = ctx.enter_context(tc.tile_pool(name="big", bufs=1))
        qp = ctx.enter_context(tc.tile_pool(name="q", bufs=1))
        fr = ctx.enter_context(tc.tile_pool(name="fld", bufs=4))
        rp = ctx.enter_context(tc.tile_pool(name="rep", bufs=16))
        sp = ctx.enter_context(tc.tile_pool(name="s", bufs=16))
        se = ctx.enter_context(tc.tile_pool(name="se", bufs=4))
        ppF = ctx.enter_context(tc.tile_pool(name="psF", bufs=4, space="PSUM"))
        ppD = ctx.enter_context(tc.tile_pool(name="psD", bufs=4, space="PSUM"))

        def psum_tile(pool, tag):
            pst = pool.tile([128, 512], f32, tag=tag, name=tag)
            return pst[:, 0:W]

        def load(tag, param, cols, dt):
            t = wp_.tile([128, cols], dt, tag=tag, name=tag)
            nc.sync.dma_start(t[:], param[:, :])
            return t

        wcv_t = load("wcv", wcv, 5 * 9 * 128, bf16)
        w15_t = load("w15", w15, 9 * 32, bf16)
        wdf_t = load("wdf", wdf, 4 * 9 * 128, bf16)
        w24_t = load("w24", w24, 9 * 3, bf16)
        b24_t = load("b24", b24, 1, f32)
        mkc_t = load("mkc", maskc, FROWS, f32)
        mbc_t = load("mbc", mbiasc, 5 * FROWS, f32)
        mq15_t = load("mq15t", mq15, QR, f32)
        mb15_t = load("mb15t", mb15, QR, f32)
        mkq_t = load("mkq", maskq, NSTEP, f32)
        mbq_t = load("mbq", mbiasq, 4 * NSTEP, f32)

        def wcv_ap(stage, k):
            return wcv_t[:, (stage * 9 + k) * 128:(stage * 9 + k + 1) * 128]

        def wdf_ap(d, k):
            return wdf_t[:, (d * 9 + k) * 128:(d * 9 + k + 1) * 128]

        # ---- x input ring ----
        xr = big.tile([128, 4, WP], bf16, tag="xring")
        for s in (0, 1, 2):
            nc.sync.dma_start(xr[0:64, s, :], xin[:, s, :])
            nc.sync.dma_start(xr[64:128, s, :], xin[:, SOFF + s, :])

        # ---- one big 64-ch tile (T1 -> T2 -> T3 in place) ----
        T = big.tile([128, SR, WP], bf16, tag="T")
        nc.gpsimd.memset(T[:, 0, :], 0.0)
        nc.gpsimd.memset(T[:, SR - 1, :], 0.0)
        nc.gpsimd.memset(T[:, 1:SR - 1, 0:1], 0.0)
        nc.gpsimd.memset(T[:, 0:SR - 1, WP - 1:WP], 0.0)
        u1 = big.tile([128, RING, WP], bf16, tag="u1")
        nc.gpsimd.memset(u1[:], 0.0)
        u2 = big.tile([128, RING, WP], bf16, tag="u2")
        nc.gpsimd.memset(u2[:], 0.0)

        def evict_resid(dst_ap, ps, mb_ap, m_ap, resid_ap):
            t = se.tile([128, W], bf16, tag="ev", name="ev")
            nc.scalar.activation(t[:], ps[:], AF.Identity, bias=mb_ap, scale=m_ap)
            nc.vector.tensor_tensor(dst_ap, t[:], resid_ap, ALU.add)

        # tap-major interleaved front matmuls for a set of wavefront jobs
        def mm_multi(jobs):
            # jobs: list of (ps, stage, src_rows, skipfn or None)
            plans = []
            for (ps, stage, src_rows, skip) in jobs:
                taps = [(k, r, c) for k, (r, c) in enumerate(RC)
                        if skip is None or skip(r)]
                plans.append((ps, stage, src_rows, taps))
            for idx in range(9):
                for (ps, stage, src_rows, taps) in plans:
                    if idx >= len(taps):
                        continue
                    k, r, c = taps[idx]
                    last = (idx == len(taps) - 1)
                    nc.tensor.matmul(
                        ps[:, :], wcv_ap(stage, k),
                        src_rows(r)[:, 1 + c:1 + c + W],
                        start=(idx == 0), stop=last,
                        skip_group_check=True)

        # ---------------- front stack, fused wavefront ----------------
        # Stage lags (0,2,4,6,8): a stage's matmuls are emitted before this
        # iteration's evictions, so readers must only touch rows evicted in
        # PRIOR iterations.
        for i in range(1, FROWS + 13):
            if 3 <= i + 1 <= FROWS + 1:
                nc.sync.dma_start(xr[0:64, (i + 1) % 4, :], xin[:, i + 1, :])
                nc.sync.dma_start(xr[64:128, (i + 1) % 4, :],
                                  xin[:, SOFF + i + 1, :])
            jobs = []
            evs = []
            if i <= FROWS:
                ps = psum_tile(ppF, "psF")
                jobs.append((ps, 0, (lambda i=i: lambda r: xr[:, (i + r) % 4, :])(),
                             None))
                evs.append(('act', ps, i, 0, T[:, i, 1:1 + W], AF.Identity))
            m = i - 3
            if 1 <= m <= FROWS:
                ps = psum_tile(ppF, "psF")
                jobs.append((ps, 1, (lambda m=m: lambda r: T[:, m + r, :])(), None))
                evs.append(('act', ps, m, 1, u1[:, m % RING, 1:1 + W], AF.Relu))
            m = i - 6
            if 1 <= m <= FROWS:
                ps = psum_tile(ppF, "psF")
                jobs.append((ps, 2,
                             (lambda m=m: lambda r: u1[:, (m + r) % RING, :])(),
                             (lambda m=m: lambda r: 1 <= m + r <= FROWS)()))
                evs.append(('resid', ps, m, 2, T[:, m, 1:1 + W], None))
            m = i - 9
            if 1 <= m <= FROWS:
                ps = psum_tile(ppF, "psF")
                jobs.append((ps, 3, (lambda m=m: lambda r: T[:, m + r, :])(), None))
                evs.append(('act', ps, m, 3, u2[:, m % RING, 1:1 + W], AF.Relu))
            m = i - 12
            if 1 <= m <= FROWS:
                ps = psum_tile(ppF, "psF")
                jobs.append((ps, 4,
                             (lambda m=m: lambda r: u2[:, (m + r) % RING, :])(),
                             (lambda m=m: lambda r: 1 <= m + r <= FROWS)()))
                evs.append(('resid', ps, m, 4, T[:, m, 1:1 + W], None))
            mm_multi(jobs)
            for (kind, ps, m, stage, dst, af) in evs:
                mb = mbc_t[:, stage * FROWS + m - 1:stage * FROWS + m]
                mk = mkc_t[:, m - 1:m]
                if kind == 'act':
                    nc.scalar.activation(dst, ps[:], af, bias=mb, scale=mk)
                else:
                    evict_resid(dst, ps, mb, mk, dst)

        # ---------------- l15: 64 -> 32 into quarter tile ----------------
        XQ = qp.tile([128, QR, WP], bf16, tag="XQ")
        nc.gpsimd.memset(XQ[:, :, 0:1], 0.0)
        nc.gpsimd.memset(XQ[:, :, WP - 1:WP], 0.0)
        for dj in range(QR):
            ps = psum_tile(ppF, "psF")
            for k, (r, c) in enumerate(RC):
                for g in range(4):
                    j = QOFF[g] + dj
                    s = 0 if j <= 45 else 1
                    rl = j - (0 if s == 0 else SOFF)
                    nc.tensor.matmul(
                        ps[32 * g:32 * g + 32, :],
                        w15_t[64 * s:64 * s + 64, k * 32:(k + 1) * 32],
                        T[64 * s:64 * s + 64, rl + r, 1 + c:1 + c + W],
                        start=(k == 0), stop=(k == 8),
                        tile_position=(64 * s, 32 * g), skip_group_check=True)
            nc.scalar.activation(XQ[:, dj, 1:1 + W], ps[:], AF.Identity,
                                 bias=mb15_t[:, dj:dj + 1],
                                 scale=mq15_t[:, dj:dj + 1])

        # ---------------- deform conv pairs (2-row steps) ----------------
        # Full-size relu tiles (28 rows) instead of rings: rows 0 and 27 stay
        # zero forever, so out-of-range taps read zeros (== skipping them) and
        # no per-tap skip logic or ring aliasing exists.
        NPE = len(PE_KS)

        def deform_rows_multi(jobs):
            # jobs co-emitted with tap loops interleaved: their PSUM tiles
            # rotate (spacing 4) so accumulation RAW latency is hidden.
            st = []
            for (d, lj0, ft, srcT, relu, dstT, nr) in jobs:
                npair = nr // 2
                pss = [ppD.tile([128, 512], f32, tag="psD", name="psD")
                       for _ in range(npair)]
                st.append({'d': d, 'lj0': lj0, 'ft': ft, 'srcT': srcT,
                           'relu': relu, 'dstT': dstT, 'nr': nr,
                           'npair': npair, 'pss': pss, 'nmm': [0] * npair})
            for k in (1, 3, 7, 0, 2, 4, 5, 6, 8):
                prods = {id(j): [] for j in st}
                for i2, (r, c) in enumerate(RC):
                    t_ = k * 9 + i2
                    sq, tg = t_ % 32, t_ // 32
                    for j in st:
                        nr = j['nr']
                        lj0 = j['lj0']
                        rep = rp.tile([128, 4, W], bf16, tag="rep", name="rep")
                        nc.vector.stream_shuffle(
                            rep[:, 0:nr, :].bitcast(f32),
                            j['ft'][:, tg, 0:nr, :].bitcast(f32), [sq] * 32)
                        prod = sp.tile([128, 4, W], bf16, tag="prod",
                                       name="prod")
                        nc.vector.tensor_tensor(
                            prod[:, 0:nr, :], rep[:, 0:nr, :],
                            j['srcT'][:, lj0 + r:lj0 + r + nr,
                                      1 + c:1 + c + W], ALU.mult)
                        prods[id(j)].append(prod)
                # same-weight matmuls grouped: one LDWEIGHTS stretch per job/k
                for j in st:
                    wap = wdf_ap(j['d'], k)
                    for prod in prods[id(j)]:
                        for pp in range(j['npair']):
                            nc.tensor.matmul(
                                j['pss'][pp][:, :], wap,
                                prod[:, 2 * pp:2 * pp + 2, :],
                                start=(j['nmm'][pp] == 0),
                                stop=(j['nmm'][pp] == 80),
                                skip_group_check=True)
                            j['nmm'][pp] += 1
            for j in st:
                for p in range(j['nr']):
                    lj = j['lj0'] + p
                    mb = mbq_t[:, j['d'] * NSTEP + lj - 1:j['d'] * NSTEP + lj]
                    mk = mkq_t[:, lj - 1:lj]
                    half = j['pss'][p // 2][:, (p % 2) * W:(p % 2 + 1) * W]
                    if j['relu']:
                        nc.scalar.activation(j['dstT'][:, lj, 1:1 + W], half,
                                             AF.Relu, bias=mb, scale=mk)
                    else:
                        evict_resid(XQ[:, lj, 1:1 + W], half, mb, mk,
                                    XQ[:, lj, 1:1 + W])

        STEPS = [(1, 4), (5, 4), (9, 4), (13, 4), (17, 4), (21, 4), (25, 2)]

        def deform_pair(d_relu, d_resid, ring, after_resid=None):
            # resid lags the relu wavefront by TWO slots so its ring reads
            # (rows up to lj0+nr) are final before co-emission.
            fts = {}
            for si in range(len(STEPS) + 2):
                jobs = []
                if si < len(STEPS):
                    lj0, nr = STEPS[si]
                    ft = fr.tile([128, 3, 4, W], bf16, tag="fld", name="fld")
                    nc.sync.dma_start(ft[:], flds[si])
                    fts[si] = ft
                    jobs.append((d_relu, lj0, ft, XQ, True, ring, nr))
                if si >= 2:
                    lj0, nr = STEPS[si - 2]
                    jobs.append((d_resid, lj0, fts.pop(si - 2), ring, False,
                                 None, nr))
                if jobs:
                    deform_rows_multi(jobs)
                    if si >= 2 and after_resid is not None:
                        after_resid(STEPS[si - 2][0] + STEPS[si - 2][1] - 1)

        def emit_l24_group(b):
            jo0 = 4 * b
            ob = se.tile([128, 4 * W], f32, tag="ob", name="ob")
            pss = []
            gs = []
            for j4 in range(4):
                jo = jo0 + j4
                g = min(jo // 16, 3)
                pss.append(psum_tile(ppF, "psF"))
                gs.append(g)
            for k, (r, c) in enumerate(RC):
                for j4 in range(4):
                    jo = jo0 + j4
                    g = gs[j4]
                    dj = jo + HALO - QOFF[g]
                    nc.tensor.matmul(
                        pss[j4][0:3, :],
                        w24_t[32 * g:32 * g + 32, k * 3:(k + 1) * 3],
                        XQ[32 * g:32 * g + 32, dj + r, 1 + c:1 + c + W],
                        start=(k == 0), stop=(k == 8),
                        tile_position=(32 * g, 0), skip_group_check=True)
            for j4 in range(4):
                nc.scalar.activation(ob[0:3, j4 * W:(j4 + 1) * W],
                                     pss[j4][0:3, :], AF.Identity,
                                     bias=b24_t[0:3, :])
            nc.sync.dma_start(
                out[:, jo0:jo0 + 4, :],
                ob[0:3, :].rearrange("p (a b) -> p a b", a=4))

        done_b = set()

        def l24_after(R):
            for b in range(16):
                if b not in done_b and 4 * (b % 4) + 10 <= R:
                    done_b.add(b)
                    emit_l24_group(b)

        r5 = qp.tile([128, QR, WP], bf16, tag="r5")
        nc.gpsimd.memset(r5[:], 0.0)
        deform_pair(0, 1, r5)
        r6 = qp.tile([128, QR, WP], bf16, tag="r6")
        nc.gpsimd.memset(r6[:], 0.0)
        deform_pair(2, 3, r6, after_resid=l24_after)
        l24_after(99)

    import concourse.mybir as mybir2
    split_excess_waits(nc, mybir2)
    return nc


# ----------------------------------------------------------------------------
# host side
# ----------------------------------------------------------------------------
def _lhsT_dup2(w, co):
    o = np.empty((9, 128, co), np.float32)
    for k, (r, c) in enumerate(RC):
        l = np.ascontiguousarray(w[:, :, r + 1, c + 1].T)
        o[k, 0:64] = l
        o[k, 64:128] = l
    return o


def _lhsT_bd2(w):
    # block-diagonal [128,128]: both 64-ch slabs in one matmul
    o = np.zeros((9, 128, 128), np.float32)
    for k, (r, c) in enumerate(RC):
        l = w[:, :, r + 1, c + 1].T
        o[k, 0:64, 0:64] = l
        o[k, 64:128, 64:128] = l
    return o


def _lhsT_dup4(w, co, grid=False):
    o = np.empty((9, 128, co), np.float32)
    for k in range(9):
        if grid:
            l = w[:, :, GY[k] + 1, GX[k] + 1].T
        else:
            r, c = RC[k]
            l = w[:, :, r + 1, c + 1].T
        for g in range(4):
            o[k, 32 * g:32 * g + 32] = l
    return o


def _flat_w(stack):
    """[S, 9, 128, co] or [9, 128, co] -> [128, S*9*co]"""
    a = np.asarray(stack, np.float32)
    if a.ndim == 3:
        a = a[None]
    return np.ascontiguousarray(a.transpose(2, 0, 1, 3).reshape(128, -1))


def _strip(a, r0, rows):
    C, H, _ = a.shape
    t = np.zeros((C, rows, WP), np.float32)
    lo, hi = max(r0, 0), min(r0 + rows, H)
    if hi > lo:
        t[:, lo - r0:hi - r0, 1:1 + W] = a[:, lo:hi]
    return t


def _prep_shards(inputs):
    x = np.asarray(inputs['x'], np.float32)
    off = np.asarray(inputs['offset_0'], np.float32)
    B, C, H, Wi = x.shape

    wcv = _flat_w(np.stack([_lhsT_bd2(np.asarray(inputs[n], np.float32))
                            for n in ('l12_w', 'l13_w1', 'l13_w2',
                                      'l14_w1', 'l14_w2')])).astype(BF)
    w15a = _flat_w(_lhsT_dup2(np.asarray(inputs['l15_w'], np.float32), 32)).astype(BF)
    def _lhsT_bd4(w):
        o = np.zeros((9, 128, 128), np.float32)
        for k in range(9):
            l = w[:, :, GY[k] + 1, GX[k] + 1].T
            for g in range(4):
                o[k, 32 * g:32 * g + 32, 32 * g:32 * g + 32] = l
        return o

    wdf = _flat_w(np.stack([_lhsT_bd4(np.asarray(inputs[n], np.float32))
                            for n in ('d50_w', 'd51_w', 'd60_w', 'd61_w')])).astype(BF)
    w24a = _flat_w(_lhsT_dup4(np.asarray(inputs['l24_w'], np.float32), 3)).astype(BF)
    b24 = np.zeros((128, 1), np.float32)
    b24[0:3, 0] = np.asarray(inputs['l24_b'], np.float32)

    fb = {k: np.asarray(inputs[k], np.float32) for k in
          ('l12_b', 'l13_b1', 'l13_b2', 'l14_b1', 'l14_b2', 'l15_b',
           'd50_b', 'd51_b', 'd60_b', 'd61_b')}

    shards = []
    for b in range(B):
        ov = off[b].reshape(12, 2, H, Wi)
        crop = ov[3:12]
        dxs_f = crop[:, 0] * RATIO
        dys_f = crop[:, 1] * RATIO
        for g4 in range(4):
            r0 = g4 * 64 - HALO

            def m(sr):
                return np.float32(1.0 if 0 <= r0 + sr < H else 0.0)

            xin = _strip(x[b], r0, ROWS).astype(BF)

            dxs = _strip(dxs_f, r0, ROWS)
            dys = _strip(dys_f, r0, ROWS)
            ax, ay = np.abs(dxs), np.abs(dys)
            wx3 = np.stack([(ax - dxs) * .5, 1 - ax, (dxs + ax) * .5])
            wy3 = np.stack([(ay - dys) * .5, 1 - ay, (dys + ay) * .5])
            fl = np.zeros((NSTEP, 128, 3, WP), np.float32)
            for k in range(9):
                for i2, (r, c) in enumerate(RC):
                    t = k * 9 + i2
                    sq, tg = t % 32, t // 32
                    P = wy3[r + 1, k] * wx3[c + 1, k]      # [ROWS, WP]
                    for qg in range(4):
                        base = QOFF[qg] + 1
                        fl[:, 32 * qg + sq, tg, :] = P[base:base + NSTEP, :]
            # per-step tile layout [7, 128, 3, 4, WP] (4-row steps, 2-row tail)
            fl4 = np.zeros((7, 128, 3, 4, W), np.float32)
            for si, (lj0, nr) in enumerate(
                    [(1, 4), (5, 4), (9, 4), (13, 4), (17, 4), (21, 4),
                     (25, 2)]):
                fl4[si, :, :, 0:nr, :] = \
                    fl[lj0 - 1:lj0 - 1 + nr, :, :, 1:1 + W].transpose(
                        1, 2, 0, 3)
            fl = fl4

            mkc = np.zeros((128, FROWS), np.float32)
            for i2 in range(1, FROWS + 1):
                mkc[0:64, i2 - 1] = m(i2)
                mkc[64:128, i2 - 1] = m(SOFF + i2)
            mbc = np.zeros((128, 5 * FROWS), np.float32)
            for si, nm in enumerate(('l12_b', 'l13_b1', 'l13_b2',
                                     'l14_b1', 'l14_b2')):
                col = np.concatenate([fb[nm], fb[nm]])
                mbc[:, si * FROWS:(si + 1) * FROWS] = mkc * col[:, None]
            mq = np.zeros((128, QR), np.float32)
            for dj in range(QR):
                for qg in range(4):
                    mq[32 * qg:32 * qg + 32, dj] = m(QOFF[qg] + dj)
            mb15v = mq * np.tile(fb['l15_b'], 4)[:, None]
            mkq = np.zeros((128, NSTEP), np.float32)
            for jj in range(NSTEP):
                for qg in range(4):
                    mkq[32 * qg:32 * qg + 32, jj] = m(QOFF[qg] + 1 + jj)
            mbq = np.zeros((128, 4 * NSTEP), np.float32)
            for di, nm in enumerate(('d50_b', 'd51_b', 'd60_b', 'd61_b')):
                mbq[:, di * NSTEP:(di + 1) * NSTEP] = \
                    mkq * np.tile(fb[nm], 4)[:, None]

            shards.append({
                'xin': xin, 'flds': fl.astype(BF), 'wcv': wcv, 'w15': w15a,
                'wdf': wdf, 'w24': w24a, 'b24': b24, 'maskc': mkc,
                'mbiasc': mbc, 'mq15': mq, 'mb15': mb15v, 'maskq': mkq,
                'mbiasq': mbq,
            })
    return shards


def kernel(**inputs):
    if 'nc' not in _cache:
        _cache['nc'] = build_nc()
    from concourse.bass_utils import run_bass_kernel_spmd
    shards = _prep_shards(inputs)
    res = run_bass_kernel_spmd(_cache['nc'], shards, core_ids=list(range(8)))
    out = np.empty((2, 3, 256, 256), np.float32)
    for i in range(8):
        b, g = divmod(i, 4)
        out[b, :, g * 64:(g + 1) * 64, :] = res.results[i]['out']
    return out
